# revision 39
# baseline (speedup 1.0000x reference)
"""Dense bilateral energy loss (DenseEnergyLoss) on 8 Trainium2 cores.

Math (per image n, after 2x downsample => oh=ow=64, P=4096):
  feat[p] = (x/40, y/40, r/15, g/15, b/15)          # 5 dims
  A[p,q]  = exp(-0.5*||feat_p - feat_q||^2)          # dense [P,P], SYMMETRIC
  loss    = -0.05 * sum_k t_k^T A u_k / (N*P)        # t = seg_m*gate, u = seg_m

Because A is symmetric, only the upper block-triangle is needed:
  loss_n = sum_{blocks i<=j} [ t_i^T B_ij u_j + (i<j) u_i^T B_ij t_j ]
A is tiled [128,512]; tile (pb, j) needed iff pb//4 <= j. The triangle is
split between 2 cores per image at 128-row granularity: core A takes even
p-blocks, core B odd ones -- both cores run the IDENTICAL program on
different flt/st data (16 virtual rows vr -> pb = 2*vr + core_parity).

The schedule runs two quad-band groups (bands 0-3, then 4-7). Each entry
(vr, bands) issues one stationary load of flt[vr] reused by up to 4 MM1s
(y = A_S*dot + B_S, Schraudolph pre-scaled, into PSUM). exp is split
across two engines:
  ACT: exact exp via the activation affine (scale=1/A_S, bias=-B_S/A_S)
  DVE: one tensor_scalar_max(0) with fp32->uint16 output conversion =
       Schraudolph bit-trick exp in bf16 (bias delta tuned numerically)
MM2 accumulates V_j (t-side) and W_j (u-side) per q-band in PSUM across
all the band's tiles; adjacent bands share one PSUM bank at partition
offsets 0/64 (tile_position). Diag tiles contribute t-only via a 21-wide
stationary. One DVE copy + DMA per band-pair evacuates [106,512]; the
host does the final masked reduction. A post-pass drops InstLdweights
that reload the identical stationary (walrus does not dedupe them).
"""

import sys

sys.path.insert(0, "/opt/trn_rl_repo")

import numpy as np
import ml_dtypes

# ---------------- problem constants (hardcoded per contract) ---------------
N, K, H, W = 4, 21, 128, 128
OH, OW = 64, 64
P = OH * OW  # 4096
WEIGHT = 0.1
SIGMA_RGB = 15.0
SIGMA_XY = 80.0
SCALE = 0.5
IGNORE_LABEL = 255
N_CORES = 8
NB = 8          # q bands of 512
QB = 512
NVR = 16        # virtual p-rows per core (128 rows each)
A_S = 128.0 / np.log(2.0)          # Schraudolph scale (bf16 bit layout)
DELTA = -7.0                       # Schraudolph bias correction (tuned)
B_S = 16256.0 + DELTA
N_DVE = 17                         # of the 40 exp chunks, how many on DVE

BF16 = ml_dtypes.bfloat16

_PROGRAM = None  # built once per process


def _hilo(x):
    x = np.asarray(x, np.float32)
    hi = x.astype(BF16)
    lo = (x - hi.astype(np.float32)).astype(BF16)
    return hi, lo


def _entries():
    """Static per-core schedule: two quad-band groups. Each group entry is
    (vr, bands) with bands = quad members >= vr//2 (ascending: any diag
    band, vr//2, comes first). Tile (vr, b) is diagonal iff vr//2 == b."""
    groups = []
    for g in range(2):
        quad = list(range(4 * g, 4 * g + 4))
        ents = []
        for vr in range(2 * quad[-1] + 2):
            bands = [b for b in quad if b >= vr // 2]
            ents.append((vr, bands))
        if g == 0:
            # split the first two entries so their bands-[0,1] halves can
            # start while the bands-[2,3] pad zero-fills are still running
            ents = [(0, [0, 1]), (1, [0, 1]), (0, [2, 3]), (1, [2, 3])] \
                + ents[2:]
        groups.append((g, quad, ents))
    return groups


def _n_chunks():
    tot = 0
    for _, _, ents in _entries():
        for _, bands in ents:
            tot += (len(bands) + 1) // 2
    return tot


def _dve_mask():
    total = _n_chunks()  # 40
    return [((k + 1) * N_DVE) // total > (k * N_DVE) // total
            for k in range(total)]


def _patch_tile_drain():
    """This container's walrus allows only one sync wait per CTRL (Drain/Nop)
    instruction; Tile's exit drain attaches one wait per DMA-HW queue sem.
    Split the extra waits onto dedicated nops."""
    from concourse import mybir
    from concourse.tile import TileContext
    from concourse.vector_clock import ScopedClock

    if getattr(TileContext, "_drain_split_patched", False):
        return

    def _drain_and_barrier(self, tick_clock, wait_clock):
        nc = self.nc
        drain_inst = nc.sync.drain()
        wait_clock.add_sem_waits(
            drain_inst.ins, ScopedClock({None: tick_clock.global_clock})
        )
        si = drain_inst.ins.sync_info
        waits = list(si.on_wait) if si is not None else []
        if len(waits) > 1:
            del si.on_wait[1:]
            for w in waits[1:]:
                n = nc.sync.nop(nofuse=True, hint="drain_split")
                n.ins.sync_info = mybir.SyncInfo(on_wait=[w], on_update=[])
        nc.all_engine_barrier()
        popped = nc._tile_sem_poison_stack.pop()
        assert popped == self._sem_poison
        nc.clear_and_free_semaphores(list(self.sems.allocated().values()))
        nc.all_engine_barrier()

    TileContext._drain_and_barrier = _drain_and_barrier
    TileContext._drain_split_patched = True


def _split_multi_waits(nc):
    """This walrus build supports one sync-wait per instruction. Hoist extra
    waits onto dedicated same-engine nops placed right before the owner."""
    from concourse import mybir

    ctr = 0
    for fn in nc.m.functions:
        for blk in fn.blocks:
            insts = blk.instructions
            new = []
            changed = False
            for inst in insts:
                si = inst.sync_info
                if si is not None and si.on_wait is not None and len(si.on_wait) > 1:
                    waits = list(si.on_wait)
                    for w in waits[:-1]:
                        ctr += 1
                        new.append(
                            mybir.InstNoOp(
                                name=f"WSPLIT-{ctr}",
                                engine=inst.engine,
                                ins=[],
                                outs=[],
                                sync_info=mybir.SyncInfo(
                                    on_wait=[w], on_update=[]
                                ),
                                text_hint="wait_split",
                                bass_nofuse=True,
                            )
                        )
                    si.on_wait = [waits[-1]]
                    inst.sync_info = si
                    changed = True
                new.append(inst)
            if changed:
                blk.instructions = new


def _dedupe_ldweights(nc):
    """Drop InstLdweights that reload a stationary already resident in that
    region of the PE array (identical AP/perf_mode at the same tile
    position, no overlapping load in between -- matmuls don't clobber
    weights). Any sync attached to a dropped load moves to the next kept
    PE instruction."""
    from concourse import mybir

    for fn in nc.m.functions:
        for blk in fn.blocks:
            new = []
            regions = {}  # col interval -> key
            carry_w, carry_u = [], []
            for inst in blk.instructions:
                tn = type(inst).__name__
                if tn == "InstLdweights":
                    a = inst.ins[0]
                    nums = [d[1] for d in a.ap]
                    m = 1
                    for x in nums[1:]:
                        m *= x
                    tp = inst.tile_position or (0, 0)
                    iv = (tp[1], tp[1] + m)
                    key = (
                        a.memref, a.offset, repr(a.ap), repr(a.dtype),
                        repr(tp), repr(inst.perf_mode),
                        repr(inst.is_transpose),
                    )
                    if regions.get(iv) == key:
                        si = inst.sync_info
                        if si is not None:
                            carry_w += list(si.on_wait or [])
                            carry_u += list(si.on_update or [])
                        continue
                    # invalidate overlapping regions, then record this one
                    for oiv in list(regions):
                        if oiv[0] < iv[1] and iv[0] < oiv[1]:
                            del regions[oiv]
                    regions[iv] = key
                elif str(getattr(inst, "engine", "")) == "EngineType.PE":
                    if tn not in ("InstMatmult", "InstNoOp"):
                        regions = {}
                if (carry_w or carry_u) and \
                        str(getattr(inst, "engine", "")) == "EngineType.PE":
                    si = inst.sync_info
                    if si is None:
                        si = mybir.SyncInfo(on_wait=[], on_update=[])
                    si.on_wait = list(si.on_wait or []) + carry_w
                    si.on_update = list(si.on_update or []) + carry_u
                    inst.sync_info = si
                    carry_w, carry_u = [], []
                new.append(inst)
            blk.instructions = new


def _build_program():
    global _PROGRAM
    if _PROGRAM is not None:
        return _PROGRAM

    _patch_tile_drain()
    import concourse.bass as bass
    from concourse import mybir
    from concourse.tile import TileContext

    nc = bass.Bass("TRN2")
    f32 = mybir.dt.float32
    bf16 = mybir.dt.bfloat16
    u16 = mybir.dt.uint16

    # Compact inputs: 21 real contraction rows shipped (rows 21:32 zero);
    # rows 32:128 are zero-filled on-device by GPSIMD memset (engine is
    # otherwise idle, and it keeps the input DMA volume minimal).
    flt = nc.dram_tensor("flt", [32, NVR * 128], bf16, kind="ExternalInput")
    frt = nc.dram_tensor("frt", [32, P], bf16, kind="ExternalInput")
    # st[p, vr*42+k] = t[k, pb*128+p] (k<21) / u[k-21, pb*128+p] (k>=21)
    st = nc.dram_tensor("st", [128, NVR * 42], bf16, kind="ExternalInput")
    # 4 band-pair chunks; rows 0:21 V_even, 21:42 W_even, 42:63 V_odd,
    # 63:84 W_odd of bands (2c, 2c+1) for chunk c
    out = nc.dram_tensor("out", [84, 4 * QB], bf16, kind="ExternalOutput")

    groups = _entries()
    dve = _dve_mask()
    scale = float(1.0 / A_S)
    bias = float(-B_S / A_S)

    # Register the activation bias constant (float bias needs a const AP).
    _bt = nc.alloc_sbuf_tensor("const-exp-bias", [128, 1], f32)
    nc.gpsimd.memset(_bt.ap(), bias)
    nc.const_aps.aps[(f32, bias)] = _bt.ap()
    nc.all_engine_barrier()

    with TileContext(nc) as tc:
        with (
            tc.tile_pool(name="const", bufs=1) as const,
            tc.tile_pool(name="apool", bufs=6) as apool,
            tc.tile_pool(name="osb", bufs=2) as osb,
            tc.tile_pool(name="dotps", bufs=3, space="PSUM") as dotps,
            tc.tile_pool(name="outps", bufs=1, space="PSUM") as outps,
        ):
            flt_s = const.tile([128, NVR * 128], bf16)
            frt_s = const.tile([128, P], bf16)
            st_s = const.tile([128, NVR * 42], bf16)

            # Zero-fill pad rows 32:128 (no input dependency, runs at t=0).
            # 32-partition pow2-width chunks -- other shapes fail codegen.
            # First-needed pads go on DVE (fast, idle until the first exp);
            # later bands' pads on GPSIMD (slow but fully off-path).
            # The first quad-group entry reads bands 0-3 (frt cols 0:2048)
            # and flt cols 0:1024; split those pads across GPSIMD and DVE so
            # they run in parallel. Later-needed pads go last on GPSIMD.
            # (Contraction must stay 128 rows: 64-row matmuls drop the PE
            # clock to 1.2GHz via the HAM activity monitor -- measured.)
            for r in range(32, 128, 32):  # flt vr0-3 first: tiny, unblocks PE
                nc.gpsimd.memset(flt_s[r:r + 32, 0:512], 0)
            nc.gpsimd.memset(frt_s[32:64, 0:1024], 0)
            nc.gpsimd.memset(frt_s[32:64, 1024:2048], 0)
            for r in range(32, 128, 32):  # flt vr4-7
                nc.gpsimd.memset(flt_s[r:r + 32, 512:1024], 0)
            nc.vector.memset(frt_s[64:96, 0:1024], 0)
            nc.vector.memset(frt_s[96:128, 0:1024], 0)
            nc.vector.memset(frt_s[64:96, 1024:2048], 0)
            nc.vector.memset(frt_s[96:128, 1024:2048], 0)
            for r in range(32, 128, 32):
                nc.gpsimd.memset(flt_s[r:r + 32, 1024:2048], 0)
                nc.gpsimd.memset(frt_s[r:r + 32, 2048:4096], 0)
            # Compact input DMAs, first-needed columns first.
            nc.sync.dma_start(out=frt_s[0:32, 0:1024], in_=frt[:, 0:1024])
            nc.sync.dma_start(out=flt_s[0:32, 0:1024], in_=flt[:, 0:1024])
            for r in range(4):
                rs = slice(32 * r, 32 * r + 32)
                nc.sync.dma_start(out=st_s[rs, :], in_=st[rs, :])
            nc.sync.dma_start(
                out=frt_s[0:32, 1024:2048], in_=frt[:, 1024:2048]
            )
            nc.sync.dma_start(out=flt_s[0:32, 1024:], in_=flt[:, 1024:])
            nc.sync.dma_start(out=frt_s[0:32, 2048:3072], in_=frt[:, 2048:3072])
            nc.sync.dma_start(out=frt_s[0:32, 3072:], in_=frt[:, 3072:])

            ccnt = 0
            pending = None  # software pipeline: MM2s trail one entry
            band_ps = {}
            band_nwr = {}

            def emit_mm2(vr, bands, ats):
                for bi, b in enumerate(bands):
                    diag = (vr // 2 == b)
                    wcols = 21 if diag else 42
                    ps = band_ps[b]
                    po = 64 * (b % 2)
                    first = band_nwr[b] == 0
                    band_nwr[b] += 1
                    last = band_nwr[b] == 2 * b + 2
                    at = ats[bi // 2]
                    nc.tensor.matmul(
                        ps[po:po + wcols, :],
                        lhsT=st_s[:, vr * 42: vr * 42 + wcols],
                        rhs=at[:, (bi % 2) * QB: (bi % 2) * QB + QB],
                        start=first,
                        stop=last,
                        tile_position=(0, po),
                    )
                    if last and b % 2 == 1:
                        cidx = b // 2
                        ob = osb.tile([106, QB], bf16, tag=f"ob{cidx % 2}",
                                      name=f"ob{cidx}")
                        nc.vector.tensor_copy(ob, ps[0:106, :])
                        cs = slice(cidx * QB, (cidx + 1) * QB)
                        # split each row-group over two DMA kicks so the
                        # trailing descriptors spread across more queues
                        for rr, (s0, s1) in enumerate(
                            ((0, 21), (21, 42), (64, 85), (85, 106))
                        ):
                            mid = (s1 - s0) // 2
                            nc.sync.dma_start(
                                out=out[21 * rr: 21 * rr + mid, cs],
                                in_=ob[s0:s0 + mid, :],
                            )
                            nc.sync.dma_start(
                                out=out[21 * rr + mid: 21 * rr + 21, cs],
                                in_=ob[s0 + mid:s1, :],
                            )

            for g, quad, ents in groups:
                for pi in range(2):
                    bp = outps.tile([128, QB], f32, tag=f"bp{pi}",
                                    name=f"bp{g}_{pi}")
                    band_ps[quad[2 * pi]] = bp
                    band_ps[quad[2 * pi + 1]] = bp
                    band_nwr[quad[2 * pi]] = 0
                    band_nwr[quad[2 * pi + 1]] = 0
                for vr, bands in ents:
                    nch = (len(bands) + 1) // 2
                    ats = []
                    dots = []
                    for ch in range(nch):
                        cb = bands[2 * ch: 2 * ch + 2]
                        dot = dotps.tile([128, 1024], f32, tag="dot",
                                         name=f"dot{ch}")
                        for bi, b in enumerate(cb):
                            nc.tensor.matmul(
                                dot[:, bi * QB: bi * QB + QB],
                                lhsT=flt_s[:, vr * 128: (vr + 1) * 128],
                                rhs=frt_s[:, b * QB: (b + 1) * QB],
                                start=True,
                                stop=True,
                            )
                        dots.append((dot, len(cb) * QB))
                    for ch, (dot, w) in enumerate(dots):
                        at = apool.tile([128, 1024], bf16, tag="at",
                                        name=f"at{ch}")
                        if dve[ccnt]:
                            nc.vector.tensor_scalar_max(
                                at[:, :w].bitcast(u16), dot[:, :w], 0.0
                            )
                        else:
                            nc.scalar.activation(
                                at[:, :w], dot[:, :w],
                                mybir.ActivationFunctionType.Exp,
                                bias=bias, scale=scale,
                            )
                        ccnt += 1
                        ats.append(at)
                    if pending is not None:
                        emit_mm2(*pending)
                    pending = (vr, bands, ats)
            emit_mm2(*pending)

    _dedupe_ldweights(nc)
    _split_multi_waits(nc)
    _PROGRAM = nc
    return nc


def _host_prep(images, segmentations, ROIs, seg_label):
    """Resizes, gate, t/u, scaled bilateral feature rows + hi/lo split."""
    images = np.asarray(images, np.float32)
    segmentations = np.asarray(segmentations, np.float32)
    ROIs = np.asarray(ROIs, np.float32)
    seg_label = np.asarray(seg_label, np.float32)

    # nearest resize (scale 0.5, floor(dst*2)) == [::2, ::2]
    img_s = images[:, :, ::2, ::2]
    roi_s = ROIs[:, ::2, ::2]
    lab_s = seg_label[:, 0, ::2, ::2]
    # bilinear (align_corners=False, scale 0.5) == 2x2 average pooling
    s = segmentations.reshape(N, K, OH, 2, OW, 2)
    seg_s = 0.25 * (s[:, :, :, 0, :, 0] + s[:, :, :, 0, :, 1]
                    + s[:, :, :, 1, :, 0] + s[:, :, :, 1, :, 1])

    unlabel = lab_s.astype(np.int32) == IGNORE_LABEL
    gate = roi_s - seg_s.max(axis=1)
    gate = np.where(unlabel, np.float32(1.0), gate)
    gate = np.maximum(gate, 0.0).reshape(N, P)
    seg_m = (seg_s * roi_s[:, None]).reshape(N, K, P)

    sxy = SIGMA_XY * SCALE
    ys, xs = np.meshgrid(np.arange(OH, dtype=np.float32),
                         np.arange(OW, dtype=np.float32), indexing="ij")
    xy = np.stack([xs.ravel(), ys.ravel()], axis=1) / sxy
    rgb = img_s.reshape(N, 3, P).transpose(0, 2, 1) / SIGMA_RGB
    feat = np.concatenate([np.broadcast_to(xy, (N, P, 2)), rgb],
                          axis=-1).astype(np.float32)  # [N,P,5]

    # Contraction rows: fL.T @ fR == A_S*(f.f' - .5|f|^2 - .5|f'|^2) + B_S
    alpha = np.float32(np.sqrt(A_S))
    sq = np.sum(feat * feat, axis=-1)
    af = alpha * feat
    m5 = -0.5 * sq * np.float32(A_S / 16.0)
    rows_L, rows_R = [], []
    hi, lo = _hilo(af)
    for d in range(5):  # hi*hi + hi*lo + lo*hi cross terms
        rows_L += [hi[..., d], hi[..., d], lo[..., d]]
        rows_R += [hi[..., d], lo[..., d], hi[..., d]]
    m5h, m5l = _hilo(m5)
    c16 = np.full((N, P), 16.0, BF16)
    rows_L += [m5h, m5l, c16, c16]
    rows_R += [c16, c16, m5h, m5l]
    c64 = np.full((N, P), 64.0, BF16)
    c254 = np.full((N, P), 254.0, BF16)
    dR = np.full((N, P), np.float32((B_S - 16256.0) / 64.0), BF16)
    rows_L += [c64, c64]
    rows_R += [c254, dR]
    nrows = len(rows_L)  # 21
    fLT = np.zeros((N, 32, P), BF16)
    fRT = np.zeros((N, 32, P), BF16)
    fLT[:, :nrows] = np.stack(rows_L, axis=1).astype(BF16)
    fRT[:, :nrows] = np.stack(rows_R, axis=1).astype(BF16)

    t = seg_m * gate[:, None]
    u = seg_m
    t_bf = t.astype(BF16)
    u_bf = u.astype(BF16)
    return seg_m, gate, t, u, t_bf, u_bf, fLT, fRT


def _make_in_maps(t_bf, u_bf, fLT, fRT):
    in_maps = []
    for c in range(N_CORES):
        n, par = c // 2, c % 2
        flt_v = np.empty((32, NVR * 128), BF16)
        st_v = np.empty((128, NVR * 42), BF16)
        for vr in range(NVR):
            pb = 2 * vr + par
            cols = slice(pb * 128, (pb + 1) * 128)
            flt_v[:, vr * 128:(vr + 1) * 128] = fLT[n][:, cols]
            st_v[:, vr * 42: vr * 42 + 21] = t_bf[n][:, cols].T
            st_v[:, vr * 42 + 21: vr * 42 + 42] = u_bf[n][:, cols].T
        in_maps.append(
            {
                "flt": np.ascontiguousarray(flt_v),
                "frt": np.ascontiguousarray(fRT[n]),
                "st": np.ascontiguousarray(st_v),
            }
        )
    return in_maps


def _reduce_outputs(res, t, u):
    loss_tot = 0.0
    for n in range(N):
        V = np.zeros((21, P), np.float64)
        Wm = np.zeros((21, P), np.float64)
        for par in range(2):
            o = res.results[2 * n + par]["out"].astype(np.float64)
            for cidx in range(4):
                ch = o[:, cidx * QB:(cidx + 1) * QB]
                b0, b1 = 2 * cidx, 2 * cidx + 1
                V[:, b0 * QB:(b0 + 1) * QB] += ch[0:21]
                Wm[:, b0 * QB:(b0 + 1) * QB] += ch[21:42]
                V[:, b1 * QB:(b1 + 1) * QB] += ch[42:63]
                Wm[:, b1 * QB:(b1 + 1) * QB] += ch[63:84]
        # W for band 0 was never written (band 0 is diag-only): exclude it.
        loss_n = np.sum(V * u[n].astype(np.float64))
        loss_n += np.sum(Wm[:, QB:] * t[n][:, QB:].astype(np.float64))
        loss_tot += loss_n
    loss = WEIGHT * (-0.5) * loss_tot / (N * P)
    return np.array(loss, dtype=np.float32)


def kernel(images, segmentations, ROIs, seg_label):
    from concourse.bass_utils import run_bass_kernel_spmd

    seg_m, gate, t, u, t_bf, u_bf, fLT, fRT = _host_prep(
        images, segmentations, ROIs, seg_label
    )
    nc = _build_program()
    in_maps = _make_in_maps(t_bf, u_bf, fLT, fRT)
    res = run_bass_kernel_spmd(nc, in_maps, core_ids=list(range(N_CORES)))
    return _reduce_outputs(res, t, u)


# revision 43
# speedup vs baseline: 1.3325x; 1.3325x over previous
"""Dense bilateral energy loss (DenseEnergyLoss) on 8 Trainium2 cores.

Math (per image n, after 2x downsample => oh=ow=64, P=4096):
  feat[p] = (x/40, y/40, r/15, g/15, b/15)          # 5 dims
  A[p,q]  = exp(-0.5*||feat_p - feat_q||^2)          # dense [P,P], SYMMETRIC
  loss    = -0.05 * sum_k t_k^T A u_k / (N*P)        # t = seg_m*gate, u = seg_m

Because A is symmetric, only the upper block-triangle is needed:
  loss_n = sum_{blocks i<=j} [ t_i^T B_ij u_j + (i<j) u_i^T B_ij t_j ]
A is tiled [128,512]; tile (pb, j) needed iff pb//4 <= j. The triangle is
split between 2 cores per image at 128-row granularity: core A takes even
p-blocks, core B odd ones -- both cores run the IDENTICAL program on
different flt/st data (16 virtual rows vr -> pb = 2*vr + core_parity).

The schedule runs two quad-band groups (bands 0-3, then 4-7). Each entry
(vr, bands) issues one stationary load of flt[vr] reused by up to 4 MM1s
(y = A_S*dot + B_S, Schraudolph pre-scaled, into PSUM). exp is split
across two engines:
  ACT: exact exp via the activation affine (scale=1/A_S, bias=-B_S/A_S)
  DVE: one tensor_scalar_max(0) with fp32->uint16 output conversion =
       Schraudolph bit-trick exp in bf16 (bias delta tuned numerically)
MM2 accumulates V_j (t-side) and W_j (u-side) per q-band in PSUM across
all the band's tiles; adjacent bands share one PSUM bank at partition
offsets 0/64 (tile_position). Diag tiles contribute t-only via a 21-wide
stationary. One DVE copy + DMA per band-pair evacuates [106,512]; the
host does the final masked reduction. A post-pass drops InstLdweights
that reload the identical stationary (walrus does not dedupe them).
"""

import sys

sys.path.insert(0, "/opt/trn_rl_repo")

import numpy as np
import ml_dtypes

# ---------------- problem constants (hardcoded per contract) ---------------
N, K, H, W = 4, 21, 128, 128
OH, OW = 64, 64
P = OH * OW  # 4096
WEIGHT = 0.1
SIGMA_RGB = 15.0
SIGMA_XY = 80.0
SCALE = 0.5
IGNORE_LABEL = 255
N_CORES = 8
NB = 8          # q bands of 512
QB = 512
NVR = 16        # virtual p-rows per core (128 rows each)
A_S = 128.0 / np.log(2.0)          # Schraudolph scale (bf16 bit layout)
DELTA = -7.0                       # Schraudolph bias correction (tuned)
B_S = 16256.0 + DELTA
N_DVE = 17                         # of the 40 exp chunks, how many on DVE

BF16 = ml_dtypes.bfloat16

_PROGRAM = None  # built once per process


def _hilo(x):
    x = np.asarray(x, np.float32)
    hi = x.astype(BF16)
    lo = (x - hi.astype(np.float32)).astype(BF16)
    return hi, lo


def _entries():
    """Static per-core schedule: two quad-band groups. Each group entry is
    (vr, bands) with bands = quad members >= vr//2 (ascending: any diag
    band, vr//2, comes first). Tile (vr, b) is diagonal iff vr//2 == b."""
    groups = []
    for g in range(2):
        quad = list(range(4 * g, 4 * g + 4))
        ents = []
        for vr in range(2 * quad[-1] + 2):
            bands = [b for b in quad if b >= vr // 2]
            ents.append((vr, bands))
        if g == 0:
            # split the first two entries so their bands-[0,1] halves can
            # start while the bands-[2,3] pad zero-fills are still running
            ents = [(0, [0, 1]), (1, [0, 1]), (0, [2, 3]), (1, [2, 3])] \
                + ents[2:]
        groups.append((g, quad, ents))
    return groups


def _n_chunks():
    tot = 0
    for _, _, ents in _entries():
        for _, bands in ents:
            tot += (len(bands) + 1) // 2
    return tot


def _dve_mask():
    total = _n_chunks()  # 40
    return [((k + 1) * N_DVE) // total > (k * N_DVE) // total
            for k in range(total)]


def _patch_tile_drain():
    """This container's walrus allows only one sync wait per CTRL (Drain/Nop)
    instruction; Tile's exit drain attaches one wait per DMA-HW queue sem.
    Split the extra waits onto dedicated nops."""
    from concourse import mybir
    from concourse.tile import TileContext
    from concourse.vector_clock import ScopedClock

    if getattr(TileContext, "_drain_split_patched", False):
        return

    def _drain_and_barrier(self, tick_clock, wait_clock):
        nc = self.nc
        drain_inst = nc.sync.drain()
        wait_clock.add_sem_waits(
            drain_inst.ins, ScopedClock({None: tick_clock.global_clock})
        )
        si = drain_inst.ins.sync_info
        waits = list(si.on_wait) if si is not None else []
        if len(waits) > 1:
            del si.on_wait[1:]
            for w in waits[1:]:
                n = nc.sync.nop(nofuse=True, hint="drain_split")
                n.ins.sync_info = mybir.SyncInfo(on_wait=[w], on_update=[])
        nc.all_engine_barrier()
        popped = nc._tile_sem_poison_stack.pop()
        assert popped == self._sem_poison
        nc.clear_and_free_semaphores(list(self.sems.allocated().values()))
        nc.all_engine_barrier()

    TileContext._drain_and_barrier = _drain_and_barrier
    TileContext._drain_split_patched = True


def _split_multi_waits(nc):
    """This walrus build supports one sync-wait per instruction. Hoist extra
    waits onto dedicated same-engine nops placed right before the owner."""
    from concourse import mybir

    ctr = 0
    for fn in nc.m.functions:
        for blk in fn.blocks:
            insts = blk.instructions
            new = []
            changed = False
            for inst in insts:
                si = inst.sync_info
                if si is not None and si.on_wait is not None and len(si.on_wait) > 1:
                    waits = list(si.on_wait)
                    for w in waits[:-1]:
                        ctr += 1
                        new.append(
                            mybir.InstNoOp(
                                name=f"WSPLIT-{ctr}",
                                engine=inst.engine,
                                ins=[],
                                outs=[],
                                sync_info=mybir.SyncInfo(
                                    on_wait=[w], on_update=[]
                                ),
                                text_hint="wait_split",
                                bass_nofuse=True,
                            )
                        )
                    si.on_wait = [waits[-1]]
                    inst.sync_info = si
                    changed = True
                new.append(inst)
            if changed:
                blk.instructions = new


def _dedupe_ldweights(nc):
    """Drop InstLdweights that reload a stationary already resident in that
    region of the PE array (identical AP/perf_mode at the same tile
    position, no overlapping load in between -- matmuls don't clobber
    weights). Any sync attached to a dropped load moves to the next kept
    PE instruction."""
    from concourse import mybir

    for fn in nc.m.functions:
        for blk in fn.blocks:
            new = []
            regions = {}  # col interval -> key
            carry_w, carry_u = [], []
            for inst in blk.instructions:
                tn = type(inst).__name__
                if tn == "InstLdweights":
                    a = inst.ins[0]
                    nums = [d[1] for d in a.ap]
                    m = 1
                    for x in nums[1:]:
                        m *= x
                    tp = inst.tile_position or (0, 0)
                    iv = (tp[1], tp[1] + m)
                    key = (
                        a.memref, a.offset, repr(a.ap), repr(a.dtype),
                        repr(tp), repr(inst.perf_mode),
                        repr(inst.is_transpose),
                    )
                    if regions.get(iv) == key:
                        si = inst.sync_info
                        if si is not None:
                            carry_w += list(si.on_wait or [])
                            carry_u += list(si.on_update or [])
                        continue
                    # invalidate overlapping regions, then record this one
                    for oiv in list(regions):
                        if oiv[0] < iv[1] and iv[0] < oiv[1]:
                            del regions[oiv]
                    regions[iv] = key
                elif str(getattr(inst, "engine", "")) == "EngineType.PE":
                    if tn not in ("InstMatmult", "InstNoOp"):
                        regions = {}
                if (carry_w or carry_u) and \
                        str(getattr(inst, "engine", "")) == "EngineType.PE":
                    si = inst.sync_info
                    if si is None:
                        si = mybir.SyncInfo(on_wait=[], on_update=[])
                    si.on_wait = list(si.on_wait or []) + carry_w
                    si.on_update = list(si.on_update or []) + carry_u
                    inst.sync_info = si
                    carry_w, carry_u = [], []
                new.append(inst)
            blk.instructions = new


def _build_program():
    global _PROGRAM
    if _PROGRAM is not None:
        return _PROGRAM

    _patch_tile_drain()
    import concourse.bass as bass
    from concourse import mybir
    from concourse.tile import TileContext

    nc = bass.Bass("TRN2")
    f32 = mybir.dt.float32
    bf16 = mybir.dt.bfloat16
    u16 = mybir.dt.uint16

    # Compact inputs: 21 real contraction rows shipped (rows 21:32 zero);
    # rows 32:128 are zero-filled on-device by GPSIMD memset (engine is
    # otherwise idle, and it keeps the input DMA volume minimal).
    flt = nc.dram_tensor("flt", [32, NVR * 128], bf16, kind="ExternalInput")
    frt = nc.dram_tensor("frt", [32, P], bf16, kind="ExternalInput")
    # st[p, vr*42+k] = t[k, pb*128+p] (k<21) / u[k-21, pb*128+p] (k>=21)
    st = nc.dram_tensor("st", [128, NVR * 42], bf16, kind="ExternalInput")
    # 4 band-pair chunks; rows 0:21 V_even, 21:42 W_even, 42:63 V_odd,
    # 63:84 W_odd of bands (2c, 2c+1) for chunk c
    out = nc.dram_tensor("out", [84, 4 * QB], bf16, kind="ExternalOutput")

    groups = _entries()
    dve = _dve_mask()
    scale = float(1.0 / A_S)
    bias = float(-B_S / A_S)

    # Register the activation bias constant (float bias needs a const AP).
    _bt = nc.alloc_sbuf_tensor("const-exp-bias", [128, 1], f32)
    nc.gpsimd.memset(_bt.ap(), bias)
    nc.const_aps.aps[(f32, bias)] = _bt.ap()
    nc.all_engine_barrier()

    with TileContext(nc) as tc:
        with (
            tc.tile_pool(name="const", bufs=1) as const,
            tc.tile_pool(name="apool", bufs=6) as apool,
            tc.tile_pool(name="osb", bufs=2) as osb,
            tc.tile_pool(name="dotps", bufs=3, space="PSUM") as dotps,
            tc.tile_pool(name="outps", bufs=1, space="PSUM") as outps,
        ):
            flt_s = const.tile([128, NVR * 128], bf16)
            frt_s = const.tile([128, P], bf16)
            st_s = const.tile([128, NVR * 42], bf16)

            # Zero-fill pad rows 32:128 (no input dependency, runs at t=0).
            # 32-partition pow2-width chunks -- other shapes fail codegen.
            # First-needed pads go on DVE (fast, idle until the first exp);
            # later bands' pads on GPSIMD (slow but fully off-path).
            # The first quad-group entry reads bands 0-3 (frt cols 0:2048)
            # and flt cols 0:1024; split those pads across GPSIMD and DVE so
            # they run in parallel. Later-needed pads go last on GPSIMD.
            # (Contraction must stay 128 rows: 64-row matmuls drop the PE
            # clock to 1.2GHz via the HAM activity monitor -- measured.)
            nc.gpsimd.memset(frt_s[32:64, 0:1024], 0)
            nc.gpsimd.memset(flt_s[32:64, 0:1024], 0)
            nc.gpsimd.memset(flt_s[64:96, 0:1024], 0)
            nc.gpsimd.memset(flt_s[96:128, 0:1024], 0)
            nc.gpsimd.memset(frt_s[32:64, 1024:2048], 0)
            nc.vector.memset(frt_s[64:96, 0:1024], 0)
            nc.vector.memset(frt_s[96:128, 0:1024], 0)
            nc.vector.memset(frt_s[64:96, 1024:2048], 0)
            nc.vector.memset(frt_s[96:128, 1024:2048], 0)
            for r in range(32, 128, 32):
                nc.gpsimd.memset(flt_s[r:r + 32, 1024:2048], 0)
                nc.gpsimd.memset(frt_s[r:r + 32, 2048:4096], 0)
            # Compact input DMAs, first-needed columns first.
            nc.sync.dma_start(out=frt_s[0:32, 0:1024], in_=frt[:, 0:1024])
            nc.sync.dma_start(out=flt_s[0:32, 0:1024], in_=flt[:, 0:1024])
            for r in range(4):
                rs = slice(32 * r, 32 * r + 32)
                nc.sync.dma_start(out=st_s[rs, :], in_=st[rs, :])
            nc.sync.dma_start(
                out=frt_s[0:32, 1024:2048], in_=frt[:, 1024:2048]
            )
            nc.sync.dma_start(out=flt_s[0:32, 1024:], in_=flt[:, 1024:])
            nc.sync.dma_start(out=frt_s[0:32, 2048:3072], in_=frt[:, 2048:3072])
            nc.sync.dma_start(out=frt_s[0:32, 3072:], in_=frt[:, 3072:])

            ccnt = 0
            pending = None  # software pipeline: MM2s trail one entry
            band_ps = {}
            band_nwr = {}

            def emit_mm2(vr, bands, ats):
                for bi, b in enumerate(bands):
                    diag = (vr // 2 == b)
                    wcols = 21 if diag else 42
                    ps = band_ps[b]
                    po = 64 * (b % 2)
                    first = band_nwr[b] == 0
                    band_nwr[b] += 1
                    last = band_nwr[b] == 2 * b + 2
                    at = ats[bi // 2]
                    nc.tensor.matmul(
                        ps[po:po + wcols, :],
                        lhsT=st_s[:, vr * 42: vr * 42 + wcols],
                        rhs=at[:, (bi % 2) * QB: (bi % 2) * QB + QB],
                        start=first,
                        stop=last,
                        tile_position=(0, po),
                    )
                    if last and b % 2 == 1:
                        cidx = b // 2
                        ob = osb.tile([106, QB], bf16, tag=f"ob{cidx % 2}",
                                      name=f"ob{cidx}")
                        nc.vector.tensor_copy(ob, ps[0:106, :])
                        cs = slice(cidx * QB, (cidx + 1) * QB)
                        for rr, (s0, s1) in enumerate(
                            ((0, 21), (21, 42), (64, 85), (85, 106))
                        ):
                            nc.sync.dma_start(
                                out=out[21 * rr: 21 * rr + 21, cs],
                                in_=ob[s0:s1, :],
                            )

            for g, quad, ents in groups:
                for pi in range(2):
                    bp = outps.tile([128, QB], f32, tag=f"bp{pi}",
                                    name=f"bp{g}_{pi}")
                    band_ps[quad[2 * pi]] = bp
                    band_ps[quad[2 * pi + 1]] = bp
                    band_nwr[quad[2 * pi]] = 0
                    band_nwr[quad[2 * pi + 1]] = 0
                for vr, bands in ents:
                    nch = (len(bands) + 1) // 2
                    ats = []
                    dots = []
                    for ch in range(nch):
                        cb = bands[2 * ch: 2 * ch + 2]
                        dot = dotps.tile([128, 1024], f32, tag="dot",
                                         name=f"dot{ch}")
                        for bi, b in enumerate(cb):
                            nc.tensor.matmul(
                                dot[:, bi * QB: bi * QB + QB],
                                lhsT=flt_s[:, vr * 128: (vr + 1) * 128],
                                rhs=frt_s[:, b * QB: (b + 1) * QB],
                                start=True,
                                stop=True,
                            )
                        dots.append((dot, len(cb) * QB))
                    for ch, (dot, w) in enumerate(dots):
                        at = apool.tile([128, 1024], bf16, tag="at",
                                        name=f"at{ch}")
                        if dve[ccnt]:
                            nc.vector.tensor_scalar_max(
                                at[:, :w].bitcast(u16), dot[:, :w], 0.0
                            )
                        else:
                            nc.scalar.activation(
                                at[:, :w], dot[:, :w],
                                mybir.ActivationFunctionType.Exp,
                                bias=bias, scale=scale,
                            )
                        ccnt += 1
                        ats.append(at)
                    if pending is not None:
                        emit_mm2(*pending)
                    pending = (vr, bands, ats)
            emit_mm2(*pending)

    _dedupe_ldweights(nc)
    _split_multi_waits(nc)
    _PROGRAM = nc
    return nc


def _host_prep(images, segmentations, ROIs, seg_label):
    """Resizes, gate, t/u, scaled bilateral feature rows + hi/lo split."""
    images = np.asarray(images, np.float32)
    segmentations = np.asarray(segmentations, np.float32)
    ROIs = np.asarray(ROIs, np.float32)
    seg_label = np.asarray(seg_label, np.float32)

    # nearest resize (scale 0.5, floor(dst*2)) == [::2, ::2]
    img_s = images[:, :, ::2, ::2]
    roi_s = ROIs[:, ::2, ::2]
    lab_s = seg_label[:, 0, ::2, ::2]
    # bilinear (align_corners=False, scale 0.5) == 2x2 average pooling
    s = segmentations.reshape(N, K, OH, 2, OW, 2)
    seg_s = 0.25 * (s[:, :, :, 0, :, 0] + s[:, :, :, 0, :, 1]
                    + s[:, :, :, 1, :, 0] + s[:, :, :, 1, :, 1])

    unlabel = lab_s.astype(np.int32) == IGNORE_LABEL
    gate = roi_s - seg_s.max(axis=1)
    gate = np.where(unlabel, np.float32(1.0), gate)
    gate = np.maximum(gate, 0.0).reshape(N, P)
    seg_m = (seg_s * roi_s[:, None]).reshape(N, K, P)

    sxy = SIGMA_XY * SCALE
    ys, xs = np.meshgrid(np.arange(OH, dtype=np.float32),
                         np.arange(OW, dtype=np.float32), indexing="ij")
    xy = np.stack([xs.ravel(), ys.ravel()], axis=1) / sxy
    rgb = img_s.reshape(N, 3, P).transpose(0, 2, 1) / SIGMA_RGB
    feat = np.concatenate([np.broadcast_to(xy, (N, P, 2)), rgb],
                          axis=-1).astype(np.float32)  # [N,P,5]

    # Contraction rows: fL.T @ fR == A_S*(f.f' - .5|f|^2 - .5|f'|^2) + B_S
    alpha = np.float32(np.sqrt(A_S))
    sq = np.sum(feat * feat, axis=-1)
    af = alpha * feat
    m5 = -0.5 * sq * np.float32(A_S / 16.0)
    rows_L, rows_R = [], []
    hi, lo = _hilo(af)
    for d in range(5):  # hi*hi + hi*lo + lo*hi cross terms
        rows_L += [hi[..., d], hi[..., d], lo[..., d]]
        rows_R += [hi[..., d], lo[..., d], hi[..., d]]
    m5h, m5l = _hilo(m5)
    c16 = np.full((N, P), 16.0, BF16)
    rows_L += [m5h, m5l, c16, c16]
    rows_R += [c16, c16, m5h, m5l]
    c64 = np.full((N, P), 64.0, BF16)
    c254 = np.full((N, P), 254.0, BF16)
    dR = np.full((N, P), np.float32((B_S - 16256.0) / 64.0), BF16)
    rows_L += [c64, c64]
    rows_R += [c254, dR]
    nrows = len(rows_L)  # 21
    fLT = np.zeros((N, 32, P), BF16)
    fRT = np.zeros((N, 32, P), BF16)
    fLT[:, :nrows] = np.stack(rows_L, axis=1).astype(BF16)
    fRT[:, :nrows] = np.stack(rows_R, axis=1).astype(BF16)

    t = seg_m * gate[:, None]
    u = seg_m
    t_bf = t.astype(BF16)
    u_bf = u.astype(BF16)
    return seg_m, gate, t, u, t_bf, u_bf, fLT, fRT


def _make_in_maps(t_bf, u_bf, fLT, fRT):
    in_maps = []
    for c in range(N_CORES):
        n, par = c // 2, c % 2
        flt_v = np.empty((32, NVR * 128), BF16)
        st_v = np.empty((128, NVR * 42), BF16)
        for vr in range(NVR):
            pb = 2 * vr + par
            cols = slice(pb * 128, (pb + 1) * 128)
            flt_v[:, vr * 128:(vr + 1) * 128] = fLT[n][:, cols]
            st_v[:, vr * 42: vr * 42 + 21] = t_bf[n][:, cols].T
            st_v[:, vr * 42 + 21: vr * 42 + 42] = u_bf[n][:, cols].T
        in_maps.append(
            {
                "flt": np.ascontiguousarray(flt_v),
                "frt": np.ascontiguousarray(fRT[n]),
                "st": np.ascontiguousarray(st_v),
            }
        )
    return in_maps


def _reduce_outputs(res, t, u):
    loss_tot = 0.0
    for n in range(N):
        V = np.zeros((21, P), np.float64)
        Wm = np.zeros((21, P), np.float64)
        for par in range(2):
            o = res.results[2 * n + par]["out"].astype(np.float64)
            for cidx in range(4):
                ch = o[:, cidx * QB:(cidx + 1) * QB]
                b0, b1 = 2 * cidx, 2 * cidx + 1
                V[:, b0 * QB:(b0 + 1) * QB] += ch[0:21]
                Wm[:, b0 * QB:(b0 + 1) * QB] += ch[21:42]
                V[:, b1 * QB:(b1 + 1) * QB] += ch[42:63]
                Wm[:, b1 * QB:(b1 + 1) * QB] += ch[63:84]
        # W for band 0 was never written (band 0 is diag-only): exclude it.
        loss_n = np.sum(V * u[n].astype(np.float64))
        loss_n += np.sum(Wm[:, QB:] * t[n][:, QB:].astype(np.float64))
        loss_tot += loss_n
    loss = WEIGHT * (-0.5) * loss_tot / (N * P)
    return np.array(loss, dtype=np.float32)


def kernel(images, segmentations, ROIs, seg_label):
    from concourse.bass_utils import run_bass_kernel_spmd

    seg_m, gate, t, u, t_bf, u_bf, fLT, fRT = _host_prep(
        images, segmentations, ROIs, seg_label
    )
    nc = _build_program()
    in_maps = _make_in_maps(t_bf, u_bf, fLT, fRT)
    res = run_bass_kernel_spmd(nc, in_maps, core_ids=list(range(N_CORES)))
    return _reduce_outputs(res, t, u)


# revision 44
# speedup vs baseline: 1.3622x; 1.0223x over previous
"""Dense bilateral energy loss (DenseEnergyLoss) on 8 Trainium2 cores.

Math (per image n, after 2x downsample => oh=ow=64, P=4096):
  feat[p] = (x/40, y/40, r/15, g/15, b/15)          # 5 dims
  A[p,q]  = exp(-0.5*||feat_p - feat_q||^2)          # dense [P,P], SYMMETRIC
  loss    = -0.05 * sum_k t_k^T A u_k / (N*P)        # t = seg_m*gate, u = seg_m

Because A is symmetric, only the upper block-triangle is needed:
  loss_n = sum_{blocks i<=j} [ t_i^T B_ij u_j + (i<j) u_i^T B_ij t_j ]
A is tiled [128,512]; tile (pb, j) needed iff pb//4 <= j. The triangle is
split between 2 cores per image at 128-row granularity: core A takes even
p-blocks, core B odd ones -- both cores run the IDENTICAL program on
different flt/st data (16 virtual rows vr -> pb = 2*vr + core_parity).

The schedule runs two quad-band groups (bands 0-3, then 4-7). Each entry
(vr, bands) issues one stationary load of flt[vr] reused by up to 4 MM1s
(y = A_S*dot + B_S, Schraudolph pre-scaled, into PSUM). exp is split
across two engines:
  ACT: exact exp via the activation affine (scale=1/A_S, bias=-B_S/A_S)
  DVE: one tensor_scalar_max(0) with fp32->uint16 output conversion =
       Schraudolph bit-trick exp in bf16 (bias delta tuned numerically)
MM2 accumulates V_j (t-side) and W_j (u-side) per q-band in PSUM across
all the band's tiles; adjacent bands share one PSUM bank at partition
offsets 0/64 (tile_position). Diag tiles contribute t-only via a 21-wide
stationary. One DVE copy + DMA per band-pair evacuates [106,512]; the
host does the final masked reduction. A post-pass drops InstLdweights
that reload the identical stationary (walrus does not dedupe them).
"""

import sys

sys.path.insert(0, "/opt/trn_rl_repo")

import numpy as np
import ml_dtypes

# ---------------- problem constants (hardcoded per contract) ---------------
N, K, H, W = 4, 21, 128, 128
OH, OW = 64, 64
P = OH * OW  # 4096
WEIGHT = 0.1
SIGMA_RGB = 15.0
SIGMA_XY = 80.0
SCALE = 0.5
IGNORE_LABEL = 255
N_CORES = 8
NB = 8          # q bands of 512
QB = 512
NVR = 16        # virtual p-rows per core (128 rows each)
A_S = 128.0 / np.log(2.0)          # Schraudolph scale (bf16 bit layout)
DELTA = -7.0                       # Schraudolph bias correction (tuned)
B_S = 16256.0 + DELTA
N_DVE = 17                         # of the 40 exp chunks, how many on DVE

BF16 = ml_dtypes.bfloat16

_PROGRAM = None  # built once per process


def _hilo(x):
    x = np.asarray(x, np.float32)
    hi = x.astype(BF16)
    lo = (x - hi.astype(np.float32)).astype(BF16)
    return hi, lo


def _entries():
    """Static per-core schedule: two quad-band groups. Each group entry is
    (vr, bands) with bands = quad members >= vr//2 (ascending: any diag
    band, vr//2, comes first). Tile (vr, b) is diagonal iff vr//2 == b."""
    groups = []
    for g in range(2):
        quad = list(range(4 * g, 4 * g + 4))
        ents = []
        for vr in range(2 * quad[-1] + 2):
            bands = [b for b in quad if b >= vr // 2]
            ents.append((vr, bands))
        if g == 0:
            # split the first two entries so their bands-[0,1] halves can
            # start while the bands-[2,3] pad zero-fills are still running
            ents = [(0, [0, 1]), (1, [0, 1]), (0, [2, 3]), (1, [2, 3])] \
                + ents[2:]
        groups.append((g, quad, ents))
    return groups


def _n_chunks():
    tot = 0
    for _, _, ents in _entries():
        for _, bands in ents:
            tot += (len(bands) + 1) // 2
    return tot


def _dve_mask():
    total = _n_chunks()  # 40
    return [((k + 1) * N_DVE) // total > (k * N_DVE) // total
            for k in range(total)]


def _patch_tile_drain():
    """This container's walrus allows only one sync wait per CTRL (Drain/Nop)
    instruction; Tile's exit drain attaches one wait per DMA-HW queue sem.
    Split the extra waits onto dedicated nops."""
    from concourse import mybir
    from concourse.tile import TileContext
    from concourse.vector_clock import ScopedClock

    if getattr(TileContext, "_drain_split_patched", False):
        return

    def _drain_and_barrier(self, tick_clock, wait_clock):
        nc = self.nc
        drain_inst = nc.sync.drain()
        wait_clock.add_sem_waits(
            drain_inst.ins, ScopedClock({None: tick_clock.global_clock})
        )
        si = drain_inst.ins.sync_info
        waits = list(si.on_wait) if si is not None else []
        if len(waits) > 1:
            del si.on_wait[1:]
            for w in waits[1:]:
                n = nc.sync.nop(nofuse=True, hint="drain_split")
                n.ins.sync_info = mybir.SyncInfo(on_wait=[w], on_update=[])
        nc.all_engine_barrier()
        popped = nc._tile_sem_poison_stack.pop()
        assert popped == self._sem_poison
        nc.clear_and_free_semaphores(list(self.sems.allocated().values()))
        nc.all_engine_barrier()

    TileContext._drain_and_barrier = _drain_and_barrier
    TileContext._drain_split_patched = True


def _split_multi_waits(nc):
    """This walrus build supports one sync-wait per instruction. Hoist extra
    waits onto dedicated same-engine nops placed right before the owner."""
    from concourse import mybir

    ctr = 0
    for fn in nc.m.functions:
        for blk in fn.blocks:
            insts = blk.instructions
            new = []
            changed = False
            for inst in insts:
                si = inst.sync_info
                if si is not None and si.on_wait is not None and len(si.on_wait) > 1:
                    waits = list(si.on_wait)
                    for w in waits[:-1]:
                        ctr += 1
                        new.append(
                            mybir.InstNoOp(
                                name=f"WSPLIT-{ctr}",
                                engine=inst.engine,
                                ins=[],
                                outs=[],
                                sync_info=mybir.SyncInfo(
                                    on_wait=[w], on_update=[]
                                ),
                                text_hint="wait_split",
                                bass_nofuse=True,
                            )
                        )
                    si.on_wait = [waits[-1]]
                    inst.sync_info = si
                    changed = True
                new.append(inst)
            if changed:
                blk.instructions = new


def _dedupe_ldweights(nc):
    """Drop InstLdweights that reload a stationary already resident in that
    region of the PE array (identical AP/perf_mode at the same tile
    position, no overlapping load in between -- matmuls don't clobber
    weights). Any sync attached to a dropped load moves to the next kept
    PE instruction."""
    from concourse import mybir

    for fn in nc.m.functions:
        for blk in fn.blocks:
            new = []
            regions = {}  # col interval -> key
            carry_w, carry_u = [], []
            for inst in blk.instructions:
                tn = type(inst).__name__
                if tn == "InstLdweights":
                    a = inst.ins[0]
                    nums = [d[1] for d in a.ap]
                    m = 1
                    for x in nums[1:]:
                        m *= x
                    tp = inst.tile_position or (0, 0)
                    iv = (tp[1], tp[1] + m)
                    key = (
                        a.memref, a.offset, repr(a.ap), repr(a.dtype),
                        repr(tp), repr(inst.perf_mode),
                        repr(inst.is_transpose),
                    )
                    if regions.get(iv) == key:
                        si = inst.sync_info
                        if si is not None:
                            carry_w += list(si.on_wait or [])
                            carry_u += list(si.on_update or [])
                        continue
                    # invalidate overlapping regions, then record this one
                    for oiv in list(regions):
                        if oiv[0] < iv[1] and iv[0] < oiv[1]:
                            del regions[oiv]
                    regions[iv] = key
                elif str(getattr(inst, "engine", "")) == "EngineType.PE":
                    if tn not in ("InstMatmult", "InstNoOp"):
                        regions = {}
                if (carry_w or carry_u) and \
                        str(getattr(inst, "engine", "")) == "EngineType.PE":
                    si = inst.sync_info
                    if si is None:
                        si = mybir.SyncInfo(on_wait=[], on_update=[])
                    si.on_wait = list(si.on_wait or []) + carry_w
                    si.on_update = list(si.on_update or []) + carry_u
                    inst.sync_info = si
                    carry_w, carry_u = [], []
                new.append(inst)
            blk.instructions = new


def _build_program():
    global _PROGRAM
    if _PROGRAM is not None:
        return _PROGRAM

    _patch_tile_drain()
    import concourse.bass as bass
    from concourse import mybir
    from concourse.tile import TileContext

    nc = bass.Bass("TRN2")
    f32 = mybir.dt.float32
    bf16 = mybir.dt.bfloat16
    u16 = mybir.dt.uint16

    # Compact inputs: 21 real contraction rows shipped (rows 21:32 zero);
    # rows 32:128 are zero-filled on-device by GPSIMD memset (engine is
    # otherwise idle, and it keeps the input DMA volume minimal).
    flt = nc.dram_tensor("flt", [32, NVR * 128], bf16, kind="ExternalInput")
    frt = nc.dram_tensor("frt", [32, P], bf16, kind="ExternalInput")
    # st[p, vr*42+k] = t[k, pb*128+p] (k<21) / u[k-21, pb*128+p] (k>=21)
    st = nc.dram_tensor("st", [128, NVR * 42], bf16, kind="ExternalInput")
    # 4 band-pair chunks; rows 0:21 V_even, 21:42 W_even, 42:63 V_odd,
    # 63:84 W_odd of bands (2c, 2c+1) for chunk c
    out = nc.dram_tensor("out", [84, 4 * QB], bf16, kind="ExternalOutput")

    groups = _entries()
    dve = _dve_mask()
    scale = float(1.0 / A_S)
    bias = float(-B_S / A_S)

    # Register the activation bias constant (float bias needs a const AP).
    _bt = nc.alloc_sbuf_tensor("const-exp-bias", [128, 1], f32)
    nc.gpsimd.memset(_bt.ap(), bias)
    nc.const_aps.aps[(f32, bias)] = _bt.ap()
    nc.all_engine_barrier()

    with TileContext(nc) as tc:
        with (
            tc.tile_pool(name="const", bufs=1) as const,
            tc.tile_pool(name="apool", bufs=6) as apool,
            tc.tile_pool(name="osb", bufs=2) as osb,
            tc.tile_pool(name="dotps", bufs=3, space="PSUM") as dotps,
            tc.tile_pool(name="outps", bufs=1, space="PSUM") as outps,
        ):
            flt_s = const.tile([128, NVR * 128], bf16)
            frt_s = const.tile([128, P], bf16)
            st_s = const.tile([128, NVR * 42], bf16)

            # Zero-fill pad rows 32:128 (no input dependency, runs at t=0).
            # 32-partition pow2-width chunks -- other shapes fail codegen.
            # First-needed pads go on DVE (fast, idle until the first exp);
            # later bands' pads on GPSIMD (slow but fully off-path).
            # The first quad-group entry reads bands 0-3 (frt cols 0:2048)
            # and flt cols 0:1024; split those pads across GPSIMD and DVE so
            # they run in parallel. Later-needed pads go last on GPSIMD.
            # (Contraction must stay 128 rows: 64-row matmuls drop the PE
            # clock to 1.2GHz via the HAM activity monitor -- measured.)
            nc.gpsimd.memset(frt_s[32:64, 0:1024], 0)
            nc.gpsimd.memset(flt_s[32:64, 0:1024], 0)
            nc.gpsimd.memset(flt_s[64:96, 0:1024], 0)
            nc.gpsimd.memset(flt_s[96:128, 0:1024], 0)
            nc.gpsimd.memset(frt_s[32:64, 1024:2048], 0)
            nc.vector.memset(frt_s[64:96, 0:1024], 0)
            nc.vector.memset(frt_s[96:128, 0:1024], 0)
            nc.vector.memset(frt_s[64:96, 1024:2048], 0)
            nc.vector.memset(frt_s[96:128, 1024:2048], 0)
            for r in range(32, 128, 32):
                nc.gpsimd.memset(flt_s[r:r + 32, 1024:2048], 0)
                nc.gpsimd.memset(frt_s[r:r + 32, 2048:4096], 0)
            # Compact input DMAs, first-needed columns first.
            nc.sync.dma_start(out=frt_s[0:32, 0:1024], in_=frt[:, 0:1024])
            nc.sync.dma_start(out=flt_s[0:32, 0:1024], in_=flt[:, 0:1024])
            for r in range(4):
                rs = slice(32 * r, 32 * r + 32)
                nc.sync.dma_start(out=st_s[rs, :], in_=st[rs, :])
            nc.sync.dma_start(
                out=frt_s[0:32, 1024:2048], in_=frt[:, 1024:2048]
            )
            nc.sync.dma_start(out=flt_s[0:32, 1024:], in_=flt[:, 1024:])
            nc.sync.dma_start(out=frt_s[0:32, 2048:3072], in_=frt[:, 2048:3072])
            nc.sync.dma_start(out=frt_s[0:32, 3072:], in_=frt[:, 3072:])

            ccnt = 0
            pending = None  # software pipeline: MM2s trail one entry
            band_ps = {}
            band_nwr = {}

            def emit_mm2(vr, bands, ats):
                for bi, b in enumerate(bands):
                    diag = (vr // 2 == b)
                    wcols = 21 if diag else 42
                    ps = band_ps[b]
                    po = 64 * (b % 2)
                    first = band_nwr[b] == 0
                    band_nwr[b] += 1
                    last = band_nwr[b] == 2 * b + 2
                    at = ats[bi // 2]
                    nc.tensor.matmul(
                        ps[po:po + wcols, :],
                        lhsT=st_s[:, vr * 42: vr * 42 + wcols],
                        rhs=at[:, (bi % 2) * QB: (bi % 2) * QB + QB],
                        start=first,
                        stop=last,
                        tile_position=(0, po),
                    )
                    if last and b % 2 == 1:
                        cidx = b // 2
                        ob = osb.tile([106, QB], bf16, tag=f"ob{cidx % 2}",
                                      name=f"ob{cidx}")
                        nc.vector.tensor_copy(ob, ps[0:106, :])
                        cs = slice(cidx * QB, (cidx + 1) * QB)
                        # 2 kicks per chunk: DMA kick issue serializes on the
                        # Sync engine, so fewer/bigger kicks win at the tail
                        nc.sync.dma_start(out=out[0:42, cs], in_=ob[0:42, :])
                        nc.sync.dma_start(
                            out=out[42:84, cs], in_=ob[64:106, :]
                        )

            for g, quad, ents in groups:
                for pi in range(2):
                    bp = outps.tile([128, QB], f32, tag=f"bp{pi}",
                                    name=f"bp{g}_{pi}")
                    band_ps[quad[2 * pi]] = bp
                    band_ps[quad[2 * pi + 1]] = bp
                    band_nwr[quad[2 * pi]] = 0
                    band_nwr[quad[2 * pi + 1]] = 0
                for vr, bands in ents:
                    nch = (len(bands) + 1) // 2
                    ats = []
                    dots = []
                    for ch in range(nch):
                        cb = bands[2 * ch: 2 * ch + 2]
                        dot = dotps.tile([128, 1024], f32, tag="dot",
                                         name=f"dot{ch}")
                        for bi, b in enumerate(cb):
                            nc.tensor.matmul(
                                dot[:, bi * QB: bi * QB + QB],
                                lhsT=flt_s[:, vr * 128: (vr + 1) * 128],
                                rhs=frt_s[:, b * QB: (b + 1) * QB],
                                start=True,
                                stop=True,
                            )
                        dots.append((dot, len(cb) * QB))
                    for ch, (dot, w) in enumerate(dots):
                        at = apool.tile([128, 1024], bf16, tag="at",
                                        name=f"at{ch}")
                        if dve[ccnt]:
                            nc.vector.tensor_scalar_max(
                                at[:, :w].bitcast(u16), dot[:, :w], 0.0
                            )
                        else:
                            nc.scalar.activation(
                                at[:, :w], dot[:, :w],
                                mybir.ActivationFunctionType.Exp,
                                bias=bias, scale=scale,
                            )
                        ccnt += 1
                        ats.append(at)
                    if pending is not None:
                        emit_mm2(*pending)
                    pending = (vr, bands, ats)
            emit_mm2(*pending)

    _dedupe_ldweights(nc)
    _split_multi_waits(nc)
    _PROGRAM = nc
    return nc


def _host_prep(images, segmentations, ROIs, seg_label):
    """Resizes, gate, t/u, scaled bilateral feature rows + hi/lo split."""
    images = np.asarray(images, np.float32)
    segmentations = np.asarray(segmentations, np.float32)
    ROIs = np.asarray(ROIs, np.float32)
    seg_label = np.asarray(seg_label, np.float32)

    # nearest resize (scale 0.5, floor(dst*2)) == [::2, ::2]
    img_s = images[:, :, ::2, ::2]
    roi_s = ROIs[:, ::2, ::2]
    lab_s = seg_label[:, 0, ::2, ::2]
    # bilinear (align_corners=False, scale 0.5) == 2x2 average pooling
    s = segmentations.reshape(N, K, OH, 2, OW, 2)
    seg_s = 0.25 * (s[:, :, :, 0, :, 0] + s[:, :, :, 0, :, 1]
                    + s[:, :, :, 1, :, 0] + s[:, :, :, 1, :, 1])

    unlabel = lab_s.astype(np.int32) == IGNORE_LABEL
    gate = roi_s - seg_s.max(axis=1)
    gate = np.where(unlabel, np.float32(1.0), gate)
    gate = np.maximum(gate, 0.0).reshape(N, P)
    seg_m = (seg_s * roi_s[:, None]).reshape(N, K, P)

    sxy = SIGMA_XY * SCALE
    ys, xs = np.meshgrid(np.arange(OH, dtype=np.float32),
                         np.arange(OW, dtype=np.float32), indexing="ij")
    xy = np.stack([xs.ravel(), ys.ravel()], axis=1) / sxy
    rgb = img_s.reshape(N, 3, P).transpose(0, 2, 1) / SIGMA_RGB
    feat = np.concatenate([np.broadcast_to(xy, (N, P, 2)), rgb],
                          axis=-1).astype(np.float32)  # [N,P,5]

    # Contraction rows: fL.T @ fR == A_S*(f.f' - .5|f|^2 - .5|f'|^2) + B_S
    alpha = np.float32(np.sqrt(A_S))
    sq = np.sum(feat * feat, axis=-1)
    af = alpha * feat
    m5 = -0.5 * sq * np.float32(A_S / 16.0)
    rows_L, rows_R = [], []
    hi, lo = _hilo(af)
    for d in range(5):  # hi*hi + hi*lo + lo*hi cross terms
        rows_L += [hi[..., d], hi[..., d], lo[..., d]]
        rows_R += [hi[..., d], lo[..., d], hi[..., d]]
    m5h, m5l = _hilo(m5)
    c16 = np.full((N, P), 16.0, BF16)
    rows_L += [m5h, m5l, c16, c16]
    rows_R += [c16, c16, m5h, m5l]
    c64 = np.full((N, P), 64.0, BF16)
    c254 = np.full((N, P), 254.0, BF16)
    dR = np.full((N, P), np.float32((B_S - 16256.0) / 64.0), BF16)
    rows_L += [c64, c64]
    rows_R += [c254, dR]
    nrows = len(rows_L)  # 21
    fLT = np.zeros((N, 32, P), BF16)
    fRT = np.zeros((N, 32, P), BF16)
    fLT[:, :nrows] = np.stack(rows_L, axis=1).astype(BF16)
    fRT[:, :nrows] = np.stack(rows_R, axis=1).astype(BF16)

    t = seg_m * gate[:, None]
    u = seg_m
    t_bf = t.astype(BF16)
    u_bf = u.astype(BF16)
    return seg_m, gate, t, u, t_bf, u_bf, fLT, fRT


def _make_in_maps(t_bf, u_bf, fLT, fRT):
    in_maps = []
    for c in range(N_CORES):
        n, par = c // 2, c % 2
        flt_v = np.empty((32, NVR * 128), BF16)
        st_v = np.empty((128, NVR * 42), BF16)
        for vr in range(NVR):
            pb = 2 * vr + par
            cols = slice(pb * 128, (pb + 1) * 128)
            flt_v[:, vr * 128:(vr + 1) * 128] = fLT[n][:, cols]
            st_v[:, vr * 42: vr * 42 + 21] = t_bf[n][:, cols].T
            st_v[:, vr * 42 + 21: vr * 42 + 42] = u_bf[n][:, cols].T
        in_maps.append(
            {
                "flt": np.ascontiguousarray(flt_v),
                "frt": np.ascontiguousarray(fRT[n]),
                "st": np.ascontiguousarray(st_v),
            }
        )
    return in_maps


def _reduce_outputs(res, t, u):
    loss_tot = 0.0
    for n in range(N):
        V = np.zeros((21, P), np.float64)
        Wm = np.zeros((21, P), np.float64)
        for par in range(2):
            o = res.results[2 * n + par]["out"].astype(np.float64)
            for cidx in range(4):
                ch = o[:, cidx * QB:(cidx + 1) * QB]
                b0, b1 = 2 * cidx, 2 * cidx + 1
                V[:, b0 * QB:(b0 + 1) * QB] += ch[0:21]
                Wm[:, b0 * QB:(b0 + 1) * QB] += ch[21:42]
                V[:, b1 * QB:(b1 + 1) * QB] += ch[42:63]
                Wm[:, b1 * QB:(b1 + 1) * QB] += ch[63:84]
        # W for band 0 was never written (band 0 is diag-only): exclude it.
        loss_n = np.sum(V * u[n].astype(np.float64))
        loss_n += np.sum(Wm[:, QB:] * t[n][:, QB:].astype(np.float64))
        loss_tot += loss_n
    loss = WEIGHT * (-0.5) * loss_tot / (N * P)
    return np.array(loss, dtype=np.float32)


def kernel(images, segmentations, ROIs, seg_label):
    from concourse.bass_utils import run_bass_kernel_spmd

    seg_m, gate, t, u, t_bf, u_bf, fLT, fRT = _host_prep(
        images, segmentations, ROIs, seg_label
    )
    nc = _build_program()
    in_maps = _make_in_maps(t_bf, u_bf, fLT, fRT)
    res = run_bass_kernel_spmd(nc, in_maps, core_ids=list(range(N_CORES)))
    return _reduce_outputs(res, t, u)
